# revision 1
# baseline (speedup 1.0000x reference)
"""Trainium2 Bass kernel for nn_DataEmbedding_Stats.

Computation: rolling-window stats (window=24, replicate-padded) over
x (B,S,7) -> 35 features -> circular conv1d(k=3) -> (B,S,512).

Strategy (8 NeuronCores, data parallel over batch, 4 batches/core),
slab-pipelined so stats/relayout overlap the matmul+output phase:
 - 4 slabs of 1024 seq; per slab X [128, 279] f32 with partition
   32j + 7b + c (j = 256-seq chunk, b = local batch, c = channel),
   built from contiguous staged loads + PE transposes.
 - rolling sum/sumsq/max/min via log-doubling shifted ops, split
   DVE (sum/sq/var) + Pool (max/min); finals write bf16 into one
   STK [128, 5*279] tile (stat-major blocks).
 - mini-stats prologue computes stats at seq 4094/4095 (the circular
   wrap cols) from a 25-seq load so slab 0 has no dependency on slab 3.
 - relayout: one 3-dim-AP DMA per (stat, slab) STK -> ST2A [28, 5*4104]
   bf16 (col = 4104*t + m, m-2 = seq mod 4096); F3 [106, 2050] per
   (batch, 2048-seq pair) via one 3-dim-AP DMA per (b, tap).
 - conv as matmul: per 128 positions out[128,512] = F3 slice.T @ Wt
   (bf16), bias as ones-row contraction; PSUM -> f16 stage copies split
   DVE/ACT/Pool; f16 output DMA (host upcasts to f32).
"""

import numpy as np

try:
    import concourse.bass as bass  # noqa: F401
except ImportError:
    import sys

    for _p in ("/opt/trn_rl_repo", "/root/.axon_site/_ro/trn_rl_repo"):
        if _p not in sys.path:
            sys.path.insert(0, _p)

B, S, C, W, D = 32, 4096, 7, 24, 512
NCORES = 8
BSH = B // NCORES          # batches per core
NF = 5 * C                 # 35 features
K = 3 * NF + 1             # 106 contraction rows (ones row last)
HALO = W - 1               # 23
NSLAB = 4
SLAB = S // NSLAB          # 1024
NJ = 4                     # 256-seq chunks per slab
CH = SLAB // NJ            # 256
XW = CH + HALO             # 279
STW = 4104                 # per-stat block width in ST2A
F3W = 2180                 # F3 cols (shared width for both mm groups)
NT_P = (15, 17)            # output tiles per (batch, group)
SOFF = (0, 1920)           # output seq offset per group
MW = 25                    # mini-stats window load (seq 4071..4095)

_CACHE = {}


def _build():
    import concourse.bacc as bacc
    import concourse.tile as tile
    from concourse import mybir

    f32 = mybir.dt.float32
    bf16 = mybir.dt.bfloat16
    f16 = mybir.dt.float16
    Alu = mybir.AluOpType
    Act = mybir.ActivationFunctionType

    nc = bacc.Bacc(
        "TRN2",
        target_bir_lowering=False,
        debug=False,
        enable_asserts=False,
        num_devices=NCORES,
    )

    x_d = nc.dram_tensor("x", (BSH, C, S), f32, kind="ExternalInput")
    wt_d = nc.dram_tensor("wt", (K, D), bf16, kind="ExternalInput")
    ones_d = nc.dram_tensor("ones", (1, F3W), bf16, kind="ExternalInput")
    wrap_d = nc.dram_tensor("wrap", (28, 10), bf16, kind="ExternalInput")
    y_d = nc.dram_tensor("y", (BSH, S, D), f16, kind="ExternalOutput")

    with tile.TileContext(nc) as tc:
        with (
            tc.tile_pool(name="const", bufs=1) as pco,
            tc.tile_pool(name="xp", bufs=2) as pxx,
            tc.tile_pool(name="scr", bufs=2) as pscr,
            tc.tile_pool(name="lad", bufs=1) as plad,
            tc.tile_pool(name="stk", bufs=2) as pstk,
            tc.tile_pool(name="st2", bufs=1) as pst2,
            tc.tile_pool(name="f3p", bufs=8) as pf3,
            tc.tile_pool(name="psum", bufs=4, space="PSUM") as pps,
            tc.tile_pool(name="outp", bufs=5) as pout,
        ):
            wt = pco.tile([K, D], bf16, tag="wt")
            nc.sync.dma_start(wt[:], wt_d.ap())

            ST2A = pst2.tile([32, 5 * STW], bf16, tag="ST2A")

            def chains(Xt, E, stk, swid, base, eng_mm):
                """Rolling stats on Xt [P, E] (valid outputs at cols>=23);
                finals -> bf16 stk, stat t block at col swid*t + base + col.
                Four ladders (sum/sq/max/min) emitted round-robin with
                per-level scratch so consecutive DVE ops are independent."""
                P = Xt.shape[0]
                lad = {
                    ch: [
                        plad.tile(
                            [128, E], f32, tag=f"{ch}{l}_{E}",
                            name=f"{ch}{l}_{E}",
                        )
                        for l in range(4)
                    ]
                    for ch in "CQMN"
                }
                S24 = pscr.tile([128, E], f32, tag=f"S24_{E}")
                SQ = pscr.tile([128, E], f32, tag=f"SQ_{E}")
                T4 = pscr.tile([128, E], f32, tag=f"T4_{E}")
                VV = pscr.tile([128, E], f32, tag=f"VV_{E}")
                v = nc.vector

                def blk(t):
                    return stk[0:P, swid * t + base + HALO : swid * t + base + E]

                def tt(dst, d0, a, a0, bs, b0, op):
                    v.tensor_tensor(
                        dst[0:P, d0:E], a[0:P, a0 : a0 + E - d0],
                        bs[0:P, b0 : b0 + E - d0], op,
                    )

                # x^2 on ACT first so the sq ladder can start by round 2
                nc.scalar.square(T4[0:P, 0:E], Xt[0:P, 0:E])
                nc.scalar.copy(blk(0), Xt[0:P, HALO:E])
                C, Q, M, N = lad["C"], lad["Q"], lad["M"], lad["N"]
                shifts = (1, 3, 7, 15)
                deltas = (1, 2, 4, 8)
                for l in range(4):
                    s, dl = shifts[l], deltas[l]
                    csrc = Xt if l == 0 else C[l - 1]
                    qsrc = T4 if l == 0 else Q[l - 1]
                    msrc = Xt if l == 0 else M[l - 1]
                    nsrc = Xt if l == 0 else N[l - 1]
                    tt(C[l], s, csrc, s, csrc, s - dl, Alu.add)
                    tt(M[l], s, msrc, s, msrc, s - dl, Alu.max)
                    tt(N[l], s, nsrc, s, nsrc, s - dl, Alu.min)
                    tt(Q[l], s, qsrc, s, qsrc, s - dl, Alu.add)
                # finals: level4 = L16 (idx 3) combined with L8 (idx 2) at -16
                tt(S24, 23, C[3], 23, C[2], 7, Alu.add)
                g = eng_mm
                g.tensor_tensor(blk(2), M[3][0:P, HALO:E],
                                M[2][0:P, 7 : E - 16], Alu.max)
                g.tensor_tensor(blk(3), N[3][0:P, HALO:E],
                                N[2][0:P, 7 : E - 16], Alu.min)
                tt(SQ, 23, Q[3], 23, Q[2], 7, Alu.add)
                nc.scalar.copy(blk(1), S24[0:P, HALO:E])
                # std = sqrt(max(SQ24 - S24^2/24, 0)/23) -> bf16
                nc.scalar.activation(
                    T4[0:P, HALO:E], S24[0:P, HALO:E], Act.Square, 0.0,
                    float(W**-0.5),
                )
                tt(VV, 23, SQ, 23, T4, 23, Alu.subtract)
                nc.vector.tensor_scalar(
                    SQ[0:P, HALO:E], VV[0:P, HALO:E], 0.0, None, Alu.max
                )
                nc.scalar.activation(
                    blk(4), SQ[0:P, HALO:E], Act.Sqrt, 0.0, 1.0 / (W - 1)
                )


            # ---------------- per-slab processing
            X = [None] * NSLAB
            STK = [None] * NSLAB

            def load_slab(n):
                # x is host-transposed to (BSH, C, S): each chunk row is a
                # contiguous 279-f32 run (halo included in the read range)
                Xn = pxx.tile([128, XW], f32, tag="X", name=f"X_{n}")
                X[n] = Xn
                for j in range(NJ):
                    s0 = SLAB * n + CH * j
                    eng = (nc.gpsimd, nc.sync, nc.gpsimd, nc.scalar)[j % 4]
                    if n == 0 and j == 0:
                        eng.dma_start(
                            Xn[0:28, HALO:XW],
                            x_d.ap()[:, :, 0:CH].rearrange("b c q -> (b c) q"),
                        )
                        # replicate x[b,0,c] into halo cols 0..22
                        nc.vector.tensor_scalar(
                            Xn[0:28, 0:HALO],
                            Xn[0:28, HALO : 2 * HALO],
                            0.0,
                            Xn[0:28, HALO : HALO + 1],
                            Alu.mult,
                            Alu.add,
                        )
                    else:
                        eng.dma_start(
                            Xn[32 * j : 32 * j + 28, :],
                            x_d.ap()[
                                :, :, s0 - HALO : s0 + CH
                            ].rearrange("b c q -> (b c) q"),
                        )

            def stats_slab(n):
                stk = pstk.tile([128, 5 * XW], bf16, tag="STK", name=f"STK_{n}")
                STK[n] = stk
                chains(X[n], XW, stk, XW, 0, nc.vector)

            def relay_slab(n):
                stk = STK[n]
                # one DMA per stat: STK[32j+g, XW*t+23+q] ->
                #   ST2A[g, STW*t + 2 + 1024n + 256j + q]
                for j in range(NJ):
                    # one DMA per chunk covering all 5 stats: plain
                    # partition slice (dep-tracking safe), t-stride in cols
                    src = stk[32 * j : 32 * j + 28, :].rearrange(
                        "g (t m) -> g t m", m=XW
                    )[:, :, HALO:XW]
                    c0 = 2 + SLAB * n + CH * j
                    dst = ST2A[0:28, :].rearrange("g (t m) -> g t m", m=STW)[
                        :, :, c0 : c0 + CH
                    ]
                    eng = (nc.sync, nc.scalar)[(n + j) % 2] if n < 2 else nc.scalar
                    eng.dma_start(dst, src)
                if n == 0:
                    # high wrap: seq 0,1 -> cols STW*t + 4098..4099
                    nc.sync.dma_start(
                        ST2A[0:28, :].rearrange("g (t m) -> g t m", m=STW)[
                            :, :, S + 2 : S + 4
                        ],
                        stk[0:28, :].rearrange("g (t m) -> g t m", m=XW)[
                            :, :, HALO : HALO + 2
                        ],
                    )

            def wrap_low():
                # host-precomputed stats at seq 4094/4095 -> cols STW*t + 0..1
                nc.scalar.dma_start(
                    ST2A[0:28, :].rearrange("g (t m) -> g t m", m=STW)[:, :, 0:2],
                    wrap_d.ap().rearrange("g (t m) -> g t m", m=2),
                )

            def build_f3(b, P):
                f3 = pf3.tile([K, F3W], bf16, tag="F3", name=f"f3_{b}_{P}")
                nc.gpsimd.dma_start(f3[K - 1 : K, :], ones_d.ap())
                hw = (F3W - 2) // 2  # 1089: keep DMA elements under 4KB
                for k in range(3):
                    # F3 row 35k + 5c + t (channel-major; wt permuted to match)
                    # col q <- ST2A col SOFF[P] + q + k; group 0 only uses
                    # q<=1921 (reads past that hit pad cols, never consumed)
                    for h in range(2):
                        src = ST2A[7 * b : 7 * b + 7, :].rearrange(
                            "c (t m) -> c t m", m=STW
                        )[:, :, SOFF[P] + k + hw * h : SOFF[P] + k + hw * (h + 1)]
                        dst = f3[35 * k : 35 * k + 35, hw * h : hw * (h + 1)]
                        if P == 0:
                            eng = (nc.sync, nc.scalar)[(k + h) % 2]
                        else:
                            eng = nc.sync
                        eng.dma_start(dst, src)
                return f3

            def mm_pair(P, f3s):
                nt = NT_P[P]
                for b in range(BSH):
                    f3 = f3s[b]
                    stage = pout.tile([128, nt * D], f16, tag="stage")
                    ps = None
                    for u in range(nt):
                        if u % 2 == 0:
                            ps = pps.tile([128, 2 * D], f32, tag="ps")
                        half = u % 2
                        nc.tensor.matmul(
                            ps[:, D * half : D * (half + 1)],
                            f3[:, 128 * u + 1 : 128 * u + 129],
                            wt[:],
                            start=True,
                            stop=True,
                        )
                        if half == 1 or u == nt - 1:
                            w = D * (half + 1)
                            cdst = stage[:, D * (u - half) : D * (u - half) + w]
                            if (u // 2) % 2 == 0:
                                nc.vector.tensor_copy(cdst, ps[:, 0:w])
                            else:
                                nc.scalar.copy(cdst, ps[:, 0:w])
                    deng = (nc.sync, nc.scalar)[b % 2]
                    for lo, hi in ((0, nt // 2), (nt // 2, nt)):
                        deng.dma_start(
                            y_d.ap()[
                                b,
                                SOFF[P] + lo * 128 : SOFF[P] + hi * 128,
                                :,
                            ].rearrange("(q p) d -> p q d", p=128),
                            stage[:, lo * D : hi * D].rearrange(
                                "p (q d) -> p q d", q=hi - lo
                            ),
                        )

            # ---------------- pipeline
            wrap_low()
            load_slab(0)
            stats_slab(0)
            relay_slab(0)
            load_slab(1)
            stats_slab(1)
            relay_slab(1)
            f3s0 = [build_f3(b, 0) for b in range(BSH)]
            load_slab(2)
            stats_slab(2)
            relay_slab(2)
            load_slab(3)
            stats_slab(3)
            relay_slab(3)
            f3s1 = [build_f3(b, 1) for b in range(BSH)]
            mm_pair(0, f3s0)
            mm_pair(1, f3s1)

    nc.compile()
    return nc


def _prep_host(W_conv, b_conv):
    import ml_dtypes

    wt = np.empty((K, D), np.float32)
    wkf = np.ascontiguousarray(W_conv.transpose(2, 1, 0)).copy()  # (3, 35, 512)
    wkf[:, C : 2 * C, :] *= 1.0 / W  # fold mean = S24/24 into weights
    # row order within a tap: 5c + t (channel-major, matches F3 gather)
    wkf = wkf.reshape(3, 5, C, D).transpose(0, 2, 1, 3).reshape(3, NF, D)
    wt[: K - 1] = wkf.reshape(3 * NF, D)
    wt[K - 1] = b_conv.astype(np.float32)
    return wt.astype(ml_dtypes.bfloat16)


def _run(x, W_conv, b_conv, trace=False, **kw):
    from concourse import bass_utils

    if "nc" not in _CACHE:
        _CACHE["nc"] = _build()
    nc = _CACHE["nc"]

    wt = _prep_host(np.asarray(W_conv), np.asarray(b_conv))
    import ml_dtypes

    ones = np.ones((1, F3W), ml_dtypes.bfloat16)
    x = np.asarray(x, np.float32)
    # host stats for the circular-wrap cols (seq 4094/4095), [28, 10] per
    # core: row 7b+c, col 2t+e (t: x,sum,max,min,std; e: seq 4094+e)
    wraps = []
    for i in range(NCORES):
        wr = np.empty((BSH, C, 5, 2), np.float32)
        for b in range(BSH):
            for e in range(2):
                win = x[BSH * i + b, S - W - 1 + e : S - 1 + e, :]  # (24, 7)
                s24 = win.sum(0)
                var = np.maximum(
                    (win * win).sum(0) - s24 * s24 / W, 0.0
                ) / (W - 1)
                wr[b, :, 0, e] = x[BSH * i + b, S - 2 + e, :]
                wr[b, :, 1, e] = s24
                wr[b, :, 2, e] = win.max(0)
                wr[b, :, 3, e] = win.min(0)
                wr[b, :, 4, e] = np.sqrt(var)
        wraps.append(
            wr.reshape(BSH * C, 10).astype(ml_dtypes.bfloat16)
        )
    x = np.ascontiguousarray(x.transpose(0, 2, 1))  # (B, C, S)
    in_maps = [
        {
            "x": x[BSH * i : BSH * (i + 1)],
            "wt": wt,
            "ones": ones,
            "wrap": wraps[i],
        }
        for i in range(NCORES)
    ]
    res = bass_utils.run_bass_kernel_spmd(
        nc, in_maps, core_ids=list(range(NCORES)), trace=trace, **kw
    )
    out = np.concatenate(
        [np.asarray(r["y"], np.float32) for r in res.results], axis=0
    )
    return out, res


def kernel(x, x_mark=None, W_conv=None, b_conv=None, **_unused):
    out, _ = _run(x, W_conv, b_conv, trace=False)
    return out



# revision 3
# speedup vs baseline: 1.3662x; 1.3662x over previous
"""Trainium2 Bass kernel for nn_DataEmbedding_Stats (v2).

Computation: rolling-window stats (window=24, replicate-padded) over
x (B,S,7) -> 35 features -> circular conv1d(k=3) -> (B,S,512).

Strategy (8 NeuronCores, data parallel over batch, 4 batches/core):
 - 2 super-slabs (seq 0..2079 / 2080..4095), X loaded as bf16 via
   SWDGE cast-DMA into [112, E] tiles, partition = 28j + 7b + c.
 - rolling stats via log-doubling shifted tensor_tensor ladders,
   all in bf16 (2x DVE rate); sum/sq/max/min chains + var/std.
 - hop1: STK [112, 5*E] -> ST2A [28, 5*4104] (col = seq+2), host
   wrap stats for seq 4094/4095, high-wrap for seq 0/1.
 - hop2: F3 [106, 2048] per (batch, group): 3 tap-gathers from ST2A
   (4KB runs) + ones row (bias); group g covers pos 2048g..2048g+2047.
 - matmul weight-stationary: lhsT = wt[:,128dc:+128] (bias row 105),
   rhs = F3 slice [106, 512]; PSUM [128,1024] f32 pairs; drains
   (f32->bf16 cast) split 5:3 across ACT/DVE; output DMA per
   (b,g,dc) [128,2048] bf16 into flat y [128, 65536] (4KB runs);
   host re-assembles (b, s, d) and upcasts.
"""

import numpy as np

try:
    import concourse.bass as bass  # noqa: F401
except ImportError:
    import sys

    for _p in ("/opt/trn_rl_repo", "/root/.axon_site/_ro/trn_rl_repo"):
        if _p not in sys.path:
            sys.path.insert(0, _p)

B, S, C, W, D = 32, 4096, 7, 24, 512
NCORES = 8
BSH = B // NCORES          # batches per core
NF = 5 * C                 # 35 features
K = 3 * NF + 1             # 106 contraction rows (ones/bias row last)
HALO = W - 1               # 23
STW = 4104                 # per-stat block width in ST2A (col = seq + 2)
F3W = 2048                 # F3 cols = positions per group
NG = 2                     # output groups of 2048 positions
NDC = 4                    # d_model chunks of 128
# super-slabs: (seq base, chunk len, n chunks); E = chunk + HALO
SSPEC = ((0, 520, 4), (2080, 504, 4))

_CACHE = {}


def _build():
    import concourse.bacc as bacc
    import concourse.tile as tile
    from concourse import mybir

    f32 = mybir.dt.float32
    bf16 = mybir.dt.bfloat16
    Alu = mybir.AluOpType
    Act = mybir.ActivationFunctionType

    nc = bacc.Bacc(
        "TRN2",
        target_bir_lowering=False,
        debug=False,
        enable_asserts=False,
        num_devices=NCORES,
    )

    x_d = nc.dram_tensor("x", (BSH, C, S), f32, kind="ExternalInput")
    wt_d = nc.dram_tensor("wt", (K, D), bf16, kind="ExternalInput")
    ones_d = nc.dram_tensor("ones", (1, F3W), bf16, kind="ExternalInput")
    wrap_d = nc.dram_tensor("wrap", (28, 10), bf16, kind="ExternalInput")
    y_d = nc.dram_tensor(
        "y", (128, BSH * NG * NDC * F3W), bf16, kind="ExternalOutput"
    )

    with tile.TileContext(nc) as tc:
        with (
            tc.tile_pool(name="const", bufs=1) as pco,
            tc.tile_pool(name="xp", bufs=1) as pxx,
            tc.tile_pool(name="scr", bufs=1) as pscr,
            tc.tile_pool(name="lad", bufs=1) as plad,
            tc.tile_pool(name="stk", bufs=1) as pstk,
            tc.tile_pool(name="st2", bufs=1) as pst2,
            tc.tile_pool(name="f3p", bufs=8) as pf3,
            tc.tile_pool(name="psum", bufs=4, space="PSUM") as pps,
            tc.tile_pool(name="outp", bufs=4) as pout,
        ):
            wt = pco.tile([K, D], bf16, tag="wt")
            ST2A = pst2.tile([32, 5 * STW], bf16, tag="ST2A")
            XB = [None] * 2
            STK = [None] * 2
            F3 = [[None] * NG for _ in range(BSH)]
            STAGE = [[None] * NG for _ in range(BSH)]

            def load_x(ss):
                base, ch, nj = SSPEC[ss]
                E = ch + HALO
                Xn = pxx.tile([112, E], bf16, tag=f"X{ss}", name=f"X{ss}")
                XB[ss] = Xn
                for j in range(nj):
                    s0 = base + ch * j
                    if ss == 0 and j == 0:
                        nc.gpsimd.dma_start(
                            Xn[0:28, HALO:E],
                            x_d.ap()[:, :, 0:ch].rearrange("b c q -> (b c) q"),
                        )
                    else:
                        nc.gpsimd.dma_start(
                            Xn[28 * j : 28 * j + 28, :],
                            x_d.ap()[
                                :, :, s0 - HALO : s0 + ch
                            ].rearrange("b c q -> (b c) q"),
                        )

            def halo0():
                Xn = XB[0]
                # replicate x[b,0,c] into halo cols 0..22 (scalar2 AP must
                # be f32, so stage the column through a tiny f32 tile)
                hc = pscr.tile([28, 1], f32, tag="haloc")
                nc.vector.tensor_copy(hc[0:28, 0:1], Xn[0:28, HALO : HALO + 1])
                nc.vector.tensor_scalar(
                    Xn[0:28, 0:HALO],
                    Xn[0:28, HALO : 2 * HALO],
                    0.0,
                    hc[0:28, 0:1],
                    Alu.mult,
                    Alu.add,
                )

            shifts = (1, 3, 7, 15)
            deltas = (1, 2, 4, 8)

            def mk_lad(ss, chains):
                _, ch, _ = SSPEC[ss]
                E = ch + HALO
                return {
                    c: [
                        plad.tile([112, E], bf16, tag=f"{c}{l}_{ss}",
                                  name=f"{c}{l}_{ss}")
                        for l in range(4)
                    ]
                    for c in chains
                }

            def tt(dst, d0, a, a0, bs, b0, op, E):
                nc.vector.tensor_tensor(
                    dst[0:112, d0:E], a[0:112, a0 : a0 + E - d0],
                    bs[0:112, b0 : b0 + E - d0], op,
                )

            def chain_levels(lad, src0, op, ss):
                _, ch, _ = SSPEC[ss]
                E = ch + HALO
                for l in range(4):
                    s, dl = shifts[l], deltas[l]
                    src = src0 if l == 0 else lad[l - 1]
                    tt(lad[l], s, src, s, src, s - dl, op, E)

            def stats_sumsq(ss):
                """sum + sq chains and their finals (DVE) + SQX (ACT)."""
                _, ch, _ = SSPEC[ss]
                E = ch + HALO
                Xn = XB[ss]
                SQX = pscr.tile([112, E], bf16, tag=f"SQX{ss}", name=f"SQX{ss}")
                nc.scalar.square(SQX[0:112, 0:E], Xn[0:112, 0:E])
                lad = mk_lad(ss, "CQ")
                C_, Q_ = lad["C"], lad["Q"]
                # interleave the two chains for DVE pipeline independence
                for l in range(4):
                    s, dl = shifts[l], deltas[l]
                    csrc = Xn if l == 0 else C_[l - 1]
                    qsrc = SQX if l == 0 else Q_[l - 1]
                    tt(C_[l], s, csrc, s, csrc, s - dl, Alu.add, E)
                    tt(Q_[l], s, qsrc, s, qsrc, s - dl, Alu.add, E)
                S24 = pscr.tile([112, E], bf16, tag=f"S24_{ss}", name=f"S24_{ss}")
                SQ24 = pscr.tile([112, E], bf16, tag=f"SQ24_{ss}",
                                 name=f"SQ24_{ss}")
                tt(S24, HALO, C_[3], HALO, C_[2], 7, Alu.add, E)
                tt(SQ24, HALO, Q_[3], HALO, Q_[2], 7, Alu.add, E)
                return S24, SQ24

            def stats_rest(ss, S24, SQ24):
                """max/min chains, var/std path, stat blocks -> STK."""
                _, ch, _ = SSPEC[ss]
                E = ch + HALO
                Xn = XB[ss]
                stk = pstk.tile([112, 5 * E], bf16, tag=f"STK{ss}",
                                name=f"STK{ss}")
                STK[ss] = stk

                def blk(t):
                    return stk[0:112, E * t + HALO : E * (t + 1)]

                lad = mk_lad(ss, "MN")
                M_, N_ = lad["M"], lad["N"]
                for l in range(4):
                    s, dl = shifts[l], deltas[l]
                    msrc = Xn if l == 0 else M_[l - 1]
                    nsrc = Xn if l == 0 else N_[l - 1]
                    tt(M_[l], s, msrc, s, msrc, s - dl, Alu.max, E)
                    tt(N_[l], s, nsrc, s, nsrc, s - dl, Alu.min, E)
                nc.vector.tensor_tensor(
                    blk(2), M_[3][0:112, HALO:E], M_[2][0:112, 7 : E - 16],
                    Alu.max,
                )
                nc.vector.tensor_tensor(
                    blk(3), N_[3][0:112, HALO:E], N_[2][0:112, 7 : E - 16],
                    Alu.min,
                )
                # x and mean (raw S24; 1/24 folded into weights) blocks
                nc.vector.tensor_copy(blk(0), Xn[0:112, HALO:E])
                nc.vector.tensor_copy(blk(1), S24[0:112, HALO:E])
                # std = sqrt(max(SQ24 - S24^2/24, 0)/23)
                T4 = pscr.tile([112, E], bf16, tag=f"T4_{ss}", name=f"T4_{ss}")
                VV = pscr.tile([112, E], bf16, tag=f"VV_{ss}", name=f"VV_{ss}")
                VC = pscr.tile([112, E], bf16, tag=f"VC_{ss}", name=f"VC_{ss}")
                nc.scalar.activation(
                    T4[0:112, HALO:E], S24[0:112, HALO:E], Act.Square, 0.0,
                    float(W**-0.5),
                )
                tt(VV, HALO, SQ24, HALO, T4, HALO, Alu.subtract, E)
                nc.vector.tensor_scalar(
                    VC[0:112, HALO:E], VV[0:112, HALO:E], 0.0, None, Alu.max
                )
                nc.scalar.activation(
                    blk(4), VC[0:112, HALO:E], Act.Sqrt, 0.0, 1.0 / (W - 1)
                )

            def hop1(ss):
                base, ch, nj = SSPEC[ss]
                E = ch + HALO
                stk = STK[ss]
                for j in range(nj):
                    src = stk[28 * j : 28 * j + 28, :].rearrange(
                        "g (t m) -> g t m", m=E
                    )[:, :, HALO:E]
                    c0 = 2 + base + ch * j
                    dst = ST2A[0:28, :].rearrange("g (t m) -> g t m", m=STW)[
                        :, :, c0 : c0 + ch
                    ]
                    nc.scalar.dma_start(dst, src)

            def high_wrap():
                # seq 0,1 -> ST2A cols 4098..4099 (circular high wrap)
                nc.scalar.dma_start(
                    ST2A[0:28, :].rearrange("g (t m) -> g t m", m=STW)[
                        :, :, S + 2 : S + 4
                    ],
                    STK[0][0:28, :].rearrange(
                        "g (t m) -> g t m", m=SSPEC[0][1] + HALO
                    )[:, :, HALO : HALO + 2],
                )

            def wrap_low():
                # host stats for seq 4094/4095 -> ST2A cols 0..1
                nc.gpsimd.dma_start(
                    ST2A[0:28, :].rearrange("g (t m) -> g t m", m=STW)[:, :, 0:2],
                    wrap_d.ap().rearrange("g (t m) -> g t m", m=2),
                )

            def build_f3(b, g):
                f3 = pf3.tile([K, F3W], bf16, tag="F3", name=f"f3_{b}_{g}")
                F3[b][g] = f3
                nc.scalar.dma_start(f3[K - 1 : K, :], ones_d.ap())
                for k in range(3):
                    # F3 row 35k + 5c + t, col q <- ST2A col 2048g + k + 1 + q
                    c0 = F3W * g + k + 1
                    src = ST2A[7 * b : 7 * b + 7, :].rearrange(
                        "c (t m) -> c t m", m=STW
                    )[:, :, c0 : c0 + F3W]
                    nc.gpsimd.dma_start(f3[35 * k : 35 * k + 35, :], src)

            # drain engine rotation: 5 ACT : 3 DVE per 8
            DVE_SLOTS = (1, 4, 7)

            def mm_group(g):
                di = 0
                for b in range(BSH):
                    f3 = F3[b][g]
                    stage = pout.tile(
                        [128, NDC * F3W], bf16, tag="stage",
                        name=f"stage_{b}_{g}",
                    )
                    STAGE[b][g] = stage
                    for dc in range(NDC):
                        for ph in range(2):
                            ps = pps.tile([128, 1024], f32, tag="ps")
                            for h in range(2):
                                q0 = ph * 1024 + h * 512
                                nc.tensor.matmul(
                                    ps[:, 512 * h : 512 * (h + 1)],
                                    wt[0:K, 128 * dc : 128 * (dc + 1)],
                                    f3[0:K, q0 : q0 + 512],
                                    start=True,
                                    stop=True,
                                )
                            col = dc * F3W + ph * 1024
                            if di % 8 in DVE_SLOTS:
                                nc.vector.tensor_copy(
                                    stage[:, col : col + 1024], ps[:, 0:1024]
                                )
                            else:
                                nc.scalar.copy(
                                    stage[:, col : col + 1024], ps[:, 0:1024]
                                )
                            di += 1
                        colbase = ((b * NG + g) * NDC + dc) * F3W
                        nc.sync.dma_start(
                            y_d.ap()[:, colbase : colbase + F3W],
                            stage[:, dc * F3W : (dc + 1) * F3W],
                        )

            # ---------------- pipeline
            load_x(0)
            load_x(1)
            nc.gpsimd.dma_start(wt[:], wt_d.ap())
            wrap_low()
            halo0()
            # SS0 full stats
            S24_0, SQ24_0 = stats_sumsq(0)
            stats_rest(0, S24_0, SQ24_0)
            hop1(0)
            high_wrap()
            for b in range(BSH):
                build_f3(b, 0)
            # SS1 sum/sq chains fill the DVE gap before G0 drains are ready
            S24_1, SQ24_1 = stats_sumsq(1)
            mm_group(0)
            stats_rest(1, S24_1, SQ24_1)
            hop1(1)
            for b in range(BSH):
                build_f3(b, 1)
            mm_group(1)

    nc.compile()
    return nc


def _prep_host(W_conv, b_conv):
    import ml_dtypes

    wt = np.empty((K, D), np.float32)
    wkf = np.ascontiguousarray(W_conv.transpose(2, 1, 0)).copy()  # (3, 35, 512)
    wkf[:, C : 2 * C, :] *= 1.0 / W  # fold mean = S24/24 into weights
    # row order within a tap: 5c + t (channel-major, matches F3 gather)
    wkf = wkf.reshape(3, 5, C, D).transpose(0, 2, 1, 3).reshape(3, NF, D)
    wt[: K - 1] = wkf.reshape(3 * NF, D)
    wt[K - 1] = b_conv.astype(np.float32)
    return wt.astype(ml_dtypes.bfloat16)


def _run(x, W_conv, b_conv, trace=False, **kw):
    from concourse import bass_utils

    if "nc" not in _CACHE:
        _CACHE["nc"] = _build()
    nc = _CACHE["nc"]

    wt = _prep_host(np.asarray(W_conv), np.asarray(b_conv))
    import ml_dtypes

    ones = np.ones((1, F3W), ml_dtypes.bfloat16)
    x = np.asarray(x, np.float32)
    # host stats for the circular-wrap cols (seq 4094/4095), [28, 10] per
    # core: row 7b+c, col 2t+e (t: x,sum,max,min,std; e: seq 4094+e)
    wraps = []
    for i in range(NCORES):
        wr = np.empty((BSH, C, 5, 2), np.float32)
        for b in range(BSH):
            for e in range(2):
                win = x[BSH * i + b, S - W - 1 + e : S - 1 + e, :]  # (24, 7)
                s24 = win.sum(0)
                var = np.maximum(
                    (win * win).sum(0) - s24 * s24 / W, 0.0
                ) / (W - 1)
                wr[b, :, 0, e] = x[BSH * i + b, S - 2 + e, :]
                wr[b, :, 1, e] = s24
                wr[b, :, 2, e] = win.max(0)
                wr[b, :, 3, e] = win.min(0)
                wr[b, :, 4, e] = np.sqrt(var)
        wraps.append(
            wr.reshape(BSH * C, 10).astype(ml_dtypes.bfloat16)
        )
    xt = np.ascontiguousarray(x.transpose(0, 2, 1))  # (B, C, S)
    in_maps = [
        {
            "x": xt[BSH * i : BSH * (i + 1)],
            "wt": wt,
            "ones": ones,
            "wrap": wraps[i],
        }
        for i in range(NCORES)
    ]
    res = bass_utils.run_bass_kernel_spmd(
        nc, in_maps, core_ids=list(range(NCORES)), trace=trace, **kw
    )
    outs = []
    for r in res.results:
        arr = np.asarray(r["y"], np.float32)  # (128, 65536)
        arr = arr.reshape(128, BSH, NG, NDC, F3W)
        # out[b, g*2048 + s, dc*128 + p]
        outs.append(
            np.ascontiguousarray(arr.transpose(1, 2, 4, 3, 0)).reshape(
                BSH, S, D
            )
        )
    out = np.concatenate(outs, axis=0)
    return out, res


def kernel(x, x_mark=None, W_conv=None, b_conv=None, **_unused):
    out, _ = _run(x, W_conv, b_conv, trace=False)
    return out
